# revision 1
# baseline (speedup 1.0000x reference)
"""Trainium2 Bass kernel for masked-dropout attention-score matmul.

Computes, for q/k/v [B,H,S,D] and an int32 0/1 keep-mask [B,H,S,S]:

    out = ((q @ k^T) * sqrt(D) * 2 * mask) @ v        (2 = 1/(1-p_drop))

Strategy (8 NeuronCores, SPMD, no collectives):
  - Shard the 32 (b,h) pairs 4-per-core.
  - Per pair, compute S^T = K @ Q^T on the PE (so the second matmul can
    consume it as its moving operand without any on-chip transpose),
    apply the mask fused into the PSUM->SBUF eviction on the DVE, and
    accumulate O^T = V^T @ S'^T on the PE.
  - The scale (2*sqrt(D)) is folded into V on the host; mask values are
    shipped as fp8(0/1) bytes; Q^T/K^T/V are host-rearranged so all
    device DMAs are fully contiguous.
"""

import os
import sys

sys.path.insert(0, "/opt/trn_rl_repo")

import numpy as np

import concourse.bacc as bacc
import concourse.bass as bass
import concourse.mybir as mybir
import concourse.tile as tile
from concourse.bass_utils import run_bass_kernel_spmd

B, H, SQ, SK, D = 2, 16, 2048, 2048, 128
P_DROP = 0.5
SCALE = float(D) ** 0.5 / (1.0 - P_DROP)  # folded into V on the host
N_CORES = 8
PAIRS = B * H
PAIRS_PER_CORE = PAIRS // N_CORES

F32 = mybir.dt.float32
F32R = mybir.dt.float32r
FP8 = mybir.dt.float8e4
U8 = mybir.dt.uint8
BF16 = mybir.dt.bfloat16

FP8_ONE = 0x38  # float8_e4m3 encoding of 1.0

# module-level handle for test.py to inspect timing after a traced run
LAST_RESULTS = None


def emit_body(nc, tc, ot, qt, kt, v, mt, n_pairs, sq, sk, d=D, qn=512, repeat=1,
              loop_n=1, mmdt=F32R):
    """Emit the per-core program.

    APs (all on this core's DRAM):
      qt [n_pairs, d,  sq]  f32  : Q^T per pair
      kt [n_pairs, d,  sk]  f32  : K^T per pair
      v  [n_pairs, d?, ...]      : V rearranged to [128, (sk//128)*d], f32,
                                   v[p][r][c*d+j] = V[c*128+r, j] * SCALE
      mt [n_pairs, sk, sq]  u8   : mask^T as fp8 bytes (0x00 / 0x38)
      ot [n_pairs, d,  sq]  f32  : O^T output
    """
    nkc = sk // 128
    nqc = sq // qn

    import contextlib

    with contextlib.ExitStack() as ctx:
        qt_pool = ctx.enter_context(tc.tile_pool(name="qt", bufs=2))
        kt_pool = ctx.enter_context(tc.tile_pool(name="kt", bufs=2))
        v_pool = ctx.enter_context(tc.tile_pool(name="v", bufs=2))
        m_pool = ctx.enter_context(tc.tile_pool(name="m", bufs=4))
        sp_pool = ctx.enter_context(tc.tile_pool(name="sp", bufs=6))
        o_pool = ctx.enter_context(tc.tile_pool(name="o", bufs=2))
        st_pool = ctx.enter_context(tc.tile_pool(name="st", bufs=4, space="PSUM"))
        ot_pool = ctx.enter_context(tc.tile_pool(name="otp", bufs=1, space="PSUM"))

        loop_cm = tc.For_i(0, loop_n, 1) if loop_n > 1 else contextlib.nullcontext()
        with loop_cm:
          for p in [pp for _ in range(repeat) for pp in range(n_pairs)]:
            qt_t = qt_pool.tile([128, sq], mmdt)
            nc.sync.dma_start(out=qt_t[:d], in_=qt[p])
            kt_t = kt_pool.tile([128, sk], mmdt)
            nc.sync.dma_start(out=kt_t[:d], in_=kt[p])
            v_t = v_pool.tile([128, nkc * d], mmdt)
            nc.sync.dma_start(out=v_t[:], in_=v[p])

            ot_ps = ot_pool.tile([128, sq], F32)

            for kc in range(nkc):
                m_t = m_pool.tile([128, sq], U8)
                nc.sync.dma_start(out=m_t[:], in_=mt[p, kc * 128 : (kc + 1) * 128, :])

                for qc in range(nqc):
                    st = st_pool.tile([128, qn], F32)
                    nc.tensor.matmul(
                        st[:],
                        kt_t[:d, kc * 128 : (kc + 1) * 128],
                        qt_t[:d, qc * qn : (qc + 1) * qn],
                        start=True,
                        stop=True,
                    )
                    sp = sp_pool.tile([128, qn], mmdt)
                    nc.vector.tensor_mul(
                        sp[:],
                        st[:],
                        m_t[:, qc * qn : (qc + 1) * qn].bitcast(FP8),
                    )
                    nc.tensor.matmul(
                        ot_ps[:d, qc * qn : (qc + 1) * qn],
                        v_t[:, kc * d : (kc + 1) * d],
                        sp[:],
                        start=(kc == 0),
                        stop=(kc == nkc - 1),
                    )

            o_t = o_pool.tile([128, sq], F32)
            nc.scalar.copy(o_t[:d], ot_ps[:d])
            nc.sync.dma_start(out=ot[p], in_=o_t[:d])


def emit_body_v2(
    nc, tc, ot, qt, kt, v, mt, n_pairs, sq, sk, d=D, qn=512, gn=1024, fused_mod=(1, 4),
    repeat=1, loop_n=1,
):
    """Balanced-engine variant.

    Masking is split across three engines per [128, gn] score group:
      - fused path (idx % fused_mod[1] < fused_mod[0]): DVE multiplies
        PSUM f32 scores by the fp8 mask directly -> bf16 SBUF.
      - split path: ACT evicts PSUM f32 -> bf16 SBUF, GpSimd converts the
        fp8 mask -> bf16, DVE multiplies bf16 x bf16 in its 2x mode.
    Second matmul runs with bf16 moving operand at N=gn; V ships as bf16.
    """
    nkc = sk // 128
    ngc = sq // gn

    import contextlib

    with contextlib.ExitStack() as ctx:
        qt_pool = ctx.enter_context(tc.tile_pool(name="qt", bufs=2))
        kt_pool = ctx.enter_context(tc.tile_pool(name="kt", bufs=2))
        v_pool = ctx.enter_context(tc.tile_pool(name="v", bufs=2))
        m_pool = ctx.enter_context(tc.tile_pool(name="m", bufs=4))
        sp_pool = ctx.enter_context(tc.tile_pool(name="sp", bufs=6))
        se_pool = ctx.enter_context(tc.tile_pool(name="se", bufs=4))
        mb_pool = ctx.enter_context(tc.tile_pool(name="mb", bufs=4))
        o_pool = ctx.enter_context(tc.tile_pool(name="o", bufs=2))
        st_pool = ctx.enter_context(tc.tile_pool(name="st", bufs=2, space="PSUM"))
        ot_pool = ctx.enter_context(tc.tile_pool(name="otp", bufs=1, space="PSUM"))

        unit = 0
        loop_cm = tc.For_i(0, loop_n, 1) if loop_n > 1 else contextlib.nullcontext()
        with loop_cm:
          for p in [pp for _ in range(repeat) for pp in range(n_pairs)]:
            qt_t = qt_pool.tile([128, sq], F32R)
            nc.sync.dma_start(out=qt_t[:d], in_=qt[p])
            kt_t = kt_pool.tile([128, sk], F32R)
            nc.sync.dma_start(out=kt_t[:d], in_=kt[p])
            v_t = v_pool.tile([128, nkc * d], BF16)
            nc.sync.dma_start(out=v_t[:], in_=v[p])

            ot_ps = ot_pool.tile([128, sq], F32)

            for kc in range(nkc):
                m_t = m_pool.tile([128, sq], U8)
                nc.sync.dma_start(out=m_t[:], in_=mt[p, kc * 128 : (kc + 1) * 128, :])

                for g in range(ngc):
                    st = st_pool.tile([128, gn], F32)
                    for j in range(gn // qn):
                        c0 = g * gn + j * qn
                        nc.tensor.matmul(
                            st[:, j * qn : (j + 1) * qn],
                            kt_t[:d, kc * 128 : (kc + 1) * 128],
                            qt_t[:d, c0 : c0 + qn],
                            start=True,
                            stop=True,
                        )
                    m_sl = m_t[:, g * gn : (g + 1) * gn].bitcast(FP8)
                    sp = sp_pool.tile([128, gn], BF16)
                    if unit % fused_mod[1] < fused_mod[0]:
                        nc.vector.tensor_mul(sp[:], st[:], m_sl)
                    else:
                        se = se_pool.tile([128, gn], BF16)
                        nc.scalar.copy(se[:], st[:])
                        mb = mb_pool.tile([128, gn], BF16)
                        nc.gpsimd.tensor_copy(mb[:], m_sl)
                        nc.vector.tensor_mul(sp[:], se[:], mb[:])
                    unit += 1
                    for j in range(gn // qn):
                        c0 = g * gn + j * qn
                        nc.tensor.matmul(
                            ot_ps[:d, c0 : c0 + qn],
                            v_t[:, kc * d : (kc + 1) * d],
                            sp[:, j * qn : (j + 1) * qn],
                            start=(kc == 0),
                            stop=(kc == nkc - 1),
                        )

            o_t = o_pool.tile([128, sq], F32)
            nc.scalar.copy(o_t[:d], ot_ps[:d])
            nc.sync.dma_start(out=ot[p], in_=o_t[:d])


def build_nc(n_pairs=PAIRS_PER_CORE, sq=SQ, sk=SK, d=D, qn=512, variant="v1", repeat=1,
             loop_n=1):
    nc = bacc.Bacc("TRN2", target_bir_lowering=False, debug=False)
    mmdt = F32R if variant == "v1" else BF16
    vdt = mmdt
    qt = nc.declare_dram_parameter("qt", [n_pairs, d, sq], mmdt, isOutput=False)
    kt = nc.declare_dram_parameter("kt", [n_pairs, d, sk], mmdt, isOutput=False)
    v = nc.declare_dram_parameter("v", [n_pairs, 128, (sk // 128) * d], vdt, isOutput=False)
    mt = nc.declare_dram_parameter("mt", [n_pairs, sk, sq], U8, isOutput=False)
    ot = nc.declare_dram_parameter("ot", [n_pairs, d, sq], F32, isOutput=True)
    with tile.TileContext(nc) as tc:
        if variant in ("v1", "v3"):
            emit_body(nc, tc, ot, qt, kt, v, mt, n_pairs, sq, sk, d, qn, repeat=repeat,
                      loop_n=loop_n, mmdt=mmdt)
        else:
            emit_body_v2(nc, tc, ot, qt, kt, v, mt, n_pairs, sq, sk, d, qn,
                         repeat=repeat, loop_n=loop_n)
    nc.compile()
    return nc


def _prep_inputs(query, key, value, dropout_mask, variant="v1"):
    """Host-side marshaling into per-core input maps."""
    import ml_dtypes

    q = np.asarray(query, dtype=np.float32).reshape(PAIRS, SQ, D)
    k = np.asarray(key, dtype=np.float32).reshape(PAIRS, SK, D)
    vv = np.asarray(value, dtype=np.float32).reshape(PAIRS, SK, D)
    m = np.asarray(dropout_mask).reshape(PAIRS, SQ, SK)

    qt = np.ascontiguousarray(q.transpose(0, 2, 1))  # [PAIRS, D, SQ]
    kt = np.ascontiguousarray(k.transpose(0, 2, 1))  # [PAIRS, D, SK]
    # V * SCALE rearranged: vr[p][r][c*D+j] = V[c*128+r, j] * SCALE
    vr = (vv * np.float32(SCALE)).reshape(PAIRS, SK // 128, 128, D)
    vr = np.ascontiguousarray(vr.transpose(0, 2, 1, 3)).reshape(PAIRS, 128, (SK // 128) * D)
    if variant != "v1":
        vr = vr.astype(ml_dtypes.bfloat16)
        qt = qt.astype(ml_dtypes.bfloat16)
        kt = kt.astype(ml_dtypes.bfloat16)
    # mask^T as fp8 bytes
    mb = (m != 0).astype(np.uint8) * np.uint8(FP8_ONE)  # [PAIRS, SQ, SK] u8
    mbt = np.ascontiguousarray(mb.transpose(0, 2, 1))  # [PAIRS, SK, SQ]

    in_maps = []
    for c in range(N_CORES):
        s = slice(c * PAIRS_PER_CORE, (c + 1) * PAIRS_PER_CORE)
        in_maps.append(
            {
                "qt": qt[s],
                "kt": kt[s],
                "v": vr[s],
                "mt": mbt[s],
            }
        )
    return in_maps


def kernel(query, key, value, dropout_mask):
    global LAST_RESULTS
    variant = os.environ.get("KERNEL_VARIANT", "v1")
    in_maps = _prep_inputs(query, key, value, dropout_mask, variant)
    nc = build_nc(variant=variant)
    res = run_bass_kernel_spmd(nc, in_maps, list(range(N_CORES)), trace=False)
    LAST_RESULTS = res
    outs = np.concatenate([r["ot"] for r in res.results], axis=0)  # [PAIRS, D, SQ]
    out = outs.transpose(0, 2, 1).reshape(B, H, SQ, D)
    return np.ascontiguousarray(out.astype(np.float32, copy=False))



# revision 11
# speedup vs baseline: 1.0615x; 1.0615x over previous
"""Trainium2 Bass kernel for masked-dropout attention-score matmul.

Computes, for q/k/v [B,H,S,D] and an int32 0/1 keep-mask [B,H,S,S]:

    out = ((q @ k^T) * sqrt(D) * 2 * mask) @ v        (2 = 1/(1-p_drop))

Strategy (8 NeuronCores, SPMD, no collectives):
  - Shard the 32 (b,h) pairs 4-per-core.
  - Per pair, compute S^T = K @ Q^T on the PE (so the second matmul can
    consume it as its moving operand without any on-chip transpose),
    apply the mask fused into the PSUM->SBUF eviction on the DVE, and
    accumulate O^T = V^T @ S'^T on the PE.
  - The scale (2*sqrt(D)) is folded into V on the host; mask values are
    shipped as fp8(0/1) bytes; Q^T/K^T/V are host-rearranged so all
    device DMAs are fully contiguous.
"""

import os
import sys

sys.path.insert(0, "/opt/trn_rl_repo")

import numpy as np

import concourse.bacc as bacc
import concourse.bass as bass
import concourse.mybir as mybir
import concourse.tile as tile
from concourse.bass_utils import run_bass_kernel_spmd

B, H, SQ, SK, D = 2, 16, 2048, 2048, 128
P_DROP = 0.5
SCALE = float(D) ** 0.5 / (1.0 - P_DROP)  # folded into V on the host
N_CORES = 8
PAIRS = B * H
PAIRS_PER_CORE = PAIRS // N_CORES

F32 = mybir.dt.float32
F32R = mybir.dt.float32r
FP8 = mybir.dt.float8e4
U8 = mybir.dt.uint8
U16 = mybir.dt.uint16
F16 = mybir.dt.float16
BF16 = mybir.dt.bfloat16

FP8_ONE = 0x38  # float8_e4m3 encoding of 1.0
FUSED_COLS = 512  # v4: q-columns masked via the DVE-fused fp8 path

# module-level handle for test.py to inspect timing after a traced run
LAST_RESULTS = None


def emit_body(nc, tc, ot, qt, kt, v, mt, n_pairs, sq, sk, d=D, qn=512, repeat=1,
              loop_n=1, mmdt=F32R):
    """Emit the per-core program.

    APs (all on this core's DRAM):
      qt [n_pairs, d,  sq]  f32  : Q^T per pair
      kt [n_pairs, d,  sk]  f32  : K^T per pair
      v  [n_pairs, d?, ...]      : V rearranged to [128, (sk//128)*d], f32,
                                   v[p][r][c*d+j] = V[c*128+r, j] * SCALE
      mt [n_pairs, sk, sq]  u8   : mask^T as fp8 bytes (0x00 / 0x38)
      ot [n_pairs, d,  sq]  f32  : O^T output
    """
    nkc = sk // 128
    nqc = sq // qn

    import contextlib

    with contextlib.ExitStack() as ctx:
        qt_pool = ctx.enter_context(tc.tile_pool(name="qt", bufs=2))
        kt_pool = ctx.enter_context(tc.tile_pool(name="kt", bufs=2))
        v_pool = ctx.enter_context(tc.tile_pool(name="v", bufs=2))
        m_pool = ctx.enter_context(tc.tile_pool(name="m", bufs=4))
        sp_pool = ctx.enter_context(tc.tile_pool(name="sp", bufs=6))
        o_pool = ctx.enter_context(tc.tile_pool(name="o", bufs=2))
        st_pool = ctx.enter_context(tc.tile_pool(name="st", bufs=4, space="PSUM"))
        ot_pool = ctx.enter_context(tc.tile_pool(name="otp", bufs=1, space="PSUM"))

        loop_cm = tc.For_i(0, loop_n, 1) if loop_n > 1 else contextlib.nullcontext()
        with loop_cm:
          for p in [pp for _ in range(repeat) for pp in range(n_pairs)]:
            qt_t = qt_pool.tile([128, sq], mmdt)
            nc.sync.dma_start(out=qt_t[:d], in_=qt[p])
            kt_t = kt_pool.tile([128, sk], mmdt)
            nc.sync.dma_start(out=kt_t[:d], in_=kt[p])
            v_t = v_pool.tile([128, nkc * d], mmdt)
            nc.sync.dma_start(out=v_t[:], in_=v[p])

            ot_ps = ot_pool.tile([128, sq], F32)

            for kc in range(nkc):
                m_t = m_pool.tile([128, sq], U8)
                nc.sync.dma_start(out=m_t[:], in_=mt[p, kc * 128 : (kc + 1) * 128, :])

                for qc in range(nqc):
                    st = st_pool.tile([128, qn], F32)
                    nc.tensor.matmul(
                        st[:],
                        kt_t[:d, kc * 128 : (kc + 1) * 128],
                        qt_t[:d, qc * qn : (qc + 1) * qn],
                        start=True,
                        stop=True,
                    )
                    sp = sp_pool.tile([128, qn], mmdt)
                    nc.vector.tensor_mul(
                        sp[:],
                        st[:],
                        m_t[:, qc * qn : (qc + 1) * qn].bitcast(FP8),
                    )
                    nc.tensor.matmul(
                        ot_ps[:d, qc * qn : (qc + 1) * qn],
                        v_t[:, kc * d : (kc + 1) * d],
                        sp[:],
                        start=(kc == 0),
                        stop=(kc == nkc - 1),
                    )

            o_t = o_pool.tile([128, sq], F32)
            nc.scalar.copy(o_t[:d], ot_ps[:d])
            nc.sync.dma_start(out=ot[p], in_=o_t[:d])


def emit_body_v2(
    nc, tc, ot, qt, kt, v, mt, n_pairs, sq, sk, d=D, qn=512, gn=1024, fused_mod=(1, 4),
    repeat=1, loop_n=1,
):
    """Balanced-engine variant.

    Masking is split across three engines per [128, gn] score group:
      - fused path (idx % fused_mod[1] < fused_mod[0]): DVE multiplies
        PSUM f32 scores by the fp8 mask directly -> bf16 SBUF.
      - split path: ACT evicts PSUM f32 -> bf16 SBUF, GpSimd converts the
        fp8 mask -> bf16, DVE multiplies bf16 x bf16 in its 2x mode.
    Second matmul runs with bf16 moving operand at N=gn; V ships as bf16.
    """
    nkc = sk // 128
    ngc = sq // gn

    import contextlib

    with contextlib.ExitStack() as ctx:
        qt_pool = ctx.enter_context(tc.tile_pool(name="qt", bufs=2))
        kt_pool = ctx.enter_context(tc.tile_pool(name="kt", bufs=2))
        v_pool = ctx.enter_context(tc.tile_pool(name="v", bufs=2))
        m_pool = ctx.enter_context(tc.tile_pool(name="m", bufs=4))
        sp_pool = ctx.enter_context(tc.tile_pool(name="sp", bufs=6))
        se_pool = ctx.enter_context(tc.tile_pool(name="se", bufs=4))
        mb_pool = ctx.enter_context(tc.tile_pool(name="mb", bufs=4))
        o_pool = ctx.enter_context(tc.tile_pool(name="o", bufs=2))
        st_pool = ctx.enter_context(tc.tile_pool(name="st", bufs=2, space="PSUM"))
        ot_pool = ctx.enter_context(tc.tile_pool(name="otp", bufs=1, space="PSUM"))

        unit = 0
        loop_cm = tc.For_i(0, loop_n, 1) if loop_n > 1 else contextlib.nullcontext()
        with loop_cm:
          for p in [pp for _ in range(repeat) for pp in range(n_pairs)]:
            qt_t = qt_pool.tile([128, sq], F32R)
            nc.sync.dma_start(out=qt_t[:d], in_=qt[p])
            kt_t = kt_pool.tile([128, sk], F32R)
            nc.sync.dma_start(out=kt_t[:d], in_=kt[p])
            v_t = v_pool.tile([128, nkc * d], BF16)
            nc.sync.dma_start(out=v_t[:], in_=v[p])

            ot_ps = ot_pool.tile([128, sq], F32)

            for kc in range(nkc):
                m_t = m_pool.tile([128, sq], U8)
                nc.sync.dma_start(out=m_t[:], in_=mt[p, kc * 128 : (kc + 1) * 128, :])

                for g in range(ngc):
                    st = st_pool.tile([128, gn], F32)
                    for j in range(gn // qn):
                        c0 = g * gn + j * qn
                        nc.tensor.matmul(
                            st[:, j * qn : (j + 1) * qn],
                            kt_t[:d, kc * 128 : (kc + 1) * 128],
                            qt_t[:d, c0 : c0 + qn],
                            start=True,
                            stop=True,
                        )
                    m_sl = m_t[:, g * gn : (g + 1) * gn].bitcast(FP8)
                    sp = sp_pool.tile([128, gn], BF16)
                    if unit % fused_mod[1] < fused_mod[0]:
                        nc.vector.tensor_mul(sp[:], st[:], m_sl)
                    else:
                        se = se_pool.tile([128, gn], BF16)
                        nc.scalar.copy(se[:], st[:])
                        mb = mb_pool.tile([128, gn], BF16)
                        nc.gpsimd.tensor_copy(mb[:], m_sl)
                        nc.vector.tensor_mul(sp[:], se[:], mb[:])
                    unit += 1
                    for j in range(gn // qn):
                        c0 = g * gn + j * qn
                        nc.tensor.matmul(
                            ot_ps[:d, c0 : c0 + qn],
                            v_t[:, kc * d : (kc + 1) * d],
                            sp[:, j * qn : (j + 1) * qn],
                            start=(kc == 0),
                            stop=(kc == nkc - 1),
                        )

            o_t = o_pool.tile([128, sq], F32)
            nc.scalar.copy(o_t[:d], ot_ps[:d])
            nc.sync.dma_start(out=ot[p], in_=o_t[:d])


def emit_body_v4(nc, tc, ot, qt, kt, v, mt8, mt16, n_pairs, sq, sk, d=D, repeat=1,
                 loop_n=1, fused_cols=512):
    """DVE-decongested variant.

    Per (pair, q-half of 1024): accumulate O^T over 16 k-chunks.
      - mm1: S^T chunk = K_chunk @ Q^T-half -> PSUM st [128, 1024] f32
      - masking splits by q-column range:
          * cols [0, fused_cols) of the full q-range: DVE fused
            tensor_mul(f32 PSUM x fp8 mask -> f16 SBUF) at 1x
          * the rest: ACT evicts PSUM -> f16 SBUF, DVE bitwise-ANDs with a
            u16 0xFFFF/0 mask at 2x (all-SBUF, 2-byte, packed)
      - mm2: O^T += V_chunk^T @ sp  (f16 moving operand)
    PSUM: ot 2 banks x2 bufs + st 2 banks x2 bufs = 8 banks.
    """
    import contextlib

    nkc = sk // 128
    hn = 1024
    nh = sq // hn

    with contextlib.ExitStack() as ctx:
        qt_pool = ctx.enter_context(tc.tile_pool(name="qt", bufs=2))
        kt_pool = ctx.enter_context(tc.tile_pool(name="kt", bufs=2))
        v_pool = ctx.enter_context(tc.tile_pool(name="v", bufs=2))
        m8_pool = ctx.enter_context(tc.tile_pool(name="m8", bufs=2))
        m16_pool = ctx.enter_context(tc.tile_pool(name="m16", bufs=2))
        se_pool = ctx.enter_context(tc.tile_pool(name="se", bufs=4))
        sp_pool = ctx.enter_context(tc.tile_pool(name="sp", bufs=4))
        o_pool = ctx.enter_context(tc.tile_pool(name="o", bufs=2))
        st_pool = ctx.enter_context(tc.tile_pool(name="st", bufs=2, space="PSUM"))
        ot_pool = ctx.enter_context(tc.tile_pool(name="otp", bufs=2, space="PSUM"))

        bw_tot = sq - fused_cols  # band-masked columns per k-chunk

        loop_cm = tc.For_i(0, loop_n, 1) if loop_n > 1 else contextlib.nullcontext()
        with loop_cm:
          for p in [pp for _ in range(repeat) for pp in range(n_pairs)]:
            qt_t = qt_pool.tile([128, sq], F16)
            nc.sync.dma_start(out=qt_t[:d], in_=qt[p])
            kt_t = kt_pool.tile([128, sk], F16)
            nc.sync.dma_start(out=kt_t[:d], in_=kt[p])
            v_t = v_pool.tile([128, nkc * d], F16)
            nc.sync.dma_start(out=v_t[:], in_=v[p])
            # whole-pair masks, chunk-major: one contiguous DMA per stream
            m8_t = m8_pool.tile([128, nkc * fused_cols], U8)
            nc.sync.dma_start(out=m8_t[:], in_=mt8[p])
            m16_t = m16_pool.tile([128, nkc * bw_tot], U16)
            nc.sync.dma_start(out=m16_t[:], in_=mt16[p])

            for h in range(nh):
                c0 = h * hn  # global q-column base of this half
                # fused (fp8-mask) columns within this half
                f_lo = min(max(fused_cols - c0, 0), hn)
                ot_ps = ot_pool.tile([128, hn], F32)

                for kc in range(nkc):
                    st = st_pool.tile([128, hn], F32)
                    for j in range(hn // 512):
                        nc.tensor.matmul(
                            st[:, j * 512 : (j + 1) * 512],
                            kt_t[:d, kc * 128 : (kc + 1) * 128],
                            qt_t[:d, c0 + j * 512 : c0 + (j + 1) * 512],
                            start=True,
                            stop=True,
                        )
                    sp = sp_pool.tile([128, hn], F16)
                    if f_lo > 0:
                        nc.vector.tensor_mul(
                            sp[:, :f_lo],
                            st[:, :f_lo],
                            m8_t[
                                :, kc * fused_cols + c0 : kc * fused_cols + c0 + f_lo
                            ].bitcast(FP8),
                        )
                    if f_lo < hn:
                        bw = hn - f_lo
                        b0 = kc * bw_tot + c0 + f_lo - fused_cols
                        se = se_pool.tile([128, bw], F16)
                        nc.scalar.copy(se[:], st[:, f_lo:hn])
                        nc.vector.tensor_tensor(
                            sp[:, f_lo:hn].bitcast(U16),
                            se[:].bitcast(U16),
                            m16_t[:, b0 : b0 + bw],
                            mybir.AluOpType.bitwise_and,
                        )
                    for j in range(hn // 512):
                        nc.tensor.matmul(
                            ot_ps[:d, j * 512 : (j + 1) * 512],
                            v_t[:, kc * d : (kc + 1) * d],
                            sp[:, j * 512 : (j + 1) * 512],
                            start=(kc == 0),
                            stop=(kc == nkc - 1),
                        )

                o_t = o_pool.tile([128, hn], F16)
                nc.scalar.copy(o_t[:d], ot_ps[:d])
                nc.sync.dma_start(out=ot[p, :, c0 : c0 + hn], in_=o_t[:d])


def build_nc(n_pairs=PAIRS_PER_CORE, sq=SQ, sk=SK, d=D, qn=512, variant="v1", repeat=1,
             loop_n=1):
    nc = bacc.Bacc("TRN2", target_bir_lowering=False, debug=False)
    if variant == "v4":
        fused_cols = FUSED_COLS
        qt = nc.declare_dram_parameter("qt", [n_pairs, d, sq], F16, isOutput=False)
        kt = nc.declare_dram_parameter("kt", [n_pairs, d, sk], F16, isOutput=False)
        v = nc.declare_dram_parameter(
            "v", [n_pairs, 128, (sk // 128) * d], F16, isOutput=False
        )
        nkc = sk // 128
        mt8 = nc.declare_dram_parameter(
            "mt8", [n_pairs, 128, nkc * fused_cols], U8, isOutput=False
        )
        mt16 = nc.declare_dram_parameter(
            "mt16", [n_pairs, 128, nkc * (sq - fused_cols)], U16, isOutput=False
        )
        ot = nc.declare_dram_parameter("ot", [n_pairs, d, sq], F16, isOutput=True)
        with tile.TileContext(nc) as tc:
            emit_body_v4(nc, tc, ot, qt, kt, v, mt8, mt16, n_pairs, sq, sk, d,
                         repeat=repeat, loop_n=loop_n, fused_cols=fused_cols)
        nc.compile()
        return nc
    mmdt = F32R if variant == "v1" else BF16
    vdt = mmdt
    qt = nc.declare_dram_parameter("qt", [n_pairs, d, sq], mmdt, isOutput=False)
    kt = nc.declare_dram_parameter("kt", [n_pairs, d, sk], mmdt, isOutput=False)
    v = nc.declare_dram_parameter("v", [n_pairs, 128, (sk // 128) * d], vdt, isOutput=False)
    mt = nc.declare_dram_parameter("mt", [n_pairs, sk, sq], U8, isOutput=False)
    ot = nc.declare_dram_parameter("ot", [n_pairs, d, sq], F32, isOutput=True)
    with tile.TileContext(nc) as tc:
        if variant in ("v1", "v3"):
            emit_body(nc, tc, ot, qt, kt, v, mt, n_pairs, sq, sk, d, qn, repeat=repeat,
                      loop_n=loop_n, mmdt=mmdt)
        else:
            emit_body_v2(nc, tc, ot, qt, kt, v, mt, n_pairs, sq, sk, d, qn,
                         repeat=repeat, loop_n=loop_n)
    nc.compile()
    return nc


def _prep_inputs(query, key, value, dropout_mask, variant="v1"):
    """Host-side marshaling into per-core input maps."""
    import ml_dtypes

    q = np.asarray(query, dtype=np.float32).reshape(PAIRS, SQ, D)
    k = np.asarray(key, dtype=np.float32).reshape(PAIRS, SK, D)
    vv = np.asarray(value, dtype=np.float32).reshape(PAIRS, SK, D)
    m = np.asarray(dropout_mask).reshape(PAIRS, SQ, SK)

    qt = np.ascontiguousarray(q.transpose(0, 2, 1))  # [PAIRS, D, SQ]
    kt = np.ascontiguousarray(k.transpose(0, 2, 1))  # [PAIRS, D, SK]
    # V * SCALE rearranged: vr[p][r][c*D+j] = V[c*128+r, j] * SCALE
    vr = (vv * np.float32(SCALE)).reshape(PAIRS, SK // 128, 128, D)
    vr = np.ascontiguousarray(vr.transpose(0, 2, 1, 3)).reshape(PAIRS, 128, (SK // 128) * D)

    if variant == "v4":
        qt = qt.astype(np.float16)
        kt = kt.astype(np.float16)
        vr = vr.astype(np.float16)
        # chunk-major mask^T: mcm[p, r, kc, c] = mask[p, q=c, k=kc*128+r]
        mcm = (m != 0).transpose(0, 2, 1).reshape(PAIRS, SK // 128, 128, SQ)
        mcm = mcm.transpose(0, 2, 1, 3)  # [PAIRS, 128, nkc, SQ]
        mt8 = np.ascontiguousarray(
            mcm[:, :, :, :FUSED_COLS]
        ).astype(np.uint8).reshape(PAIRS, 128, -1) * np.uint8(FP8_ONE)
        mt16 = np.ascontiguousarray(
            mcm[:, :, :, FUSED_COLS:]
        ).astype(np.uint16).reshape(PAIRS, 128, -1) * np.uint16(0xFFFF)
        in_maps = []
        for c in range(N_CORES):
            s = slice(c * PAIRS_PER_CORE, (c + 1) * PAIRS_PER_CORE)
            in_maps.append(
                {
                    "qt": qt[s],
                    "kt": kt[s],
                    "v": vr[s],
                    "mt8": np.ascontiguousarray(mt8[s]),
                    "mt16": np.ascontiguousarray(mt16[s]),
                }
            )
        return in_maps

    if variant != "v1":
        vr = vr.astype(ml_dtypes.bfloat16)
        qt = qt.astype(ml_dtypes.bfloat16)
        kt = kt.astype(ml_dtypes.bfloat16)
    # mask^T as fp8 bytes
    mb = (m != 0).astype(np.uint8) * np.uint8(FP8_ONE)  # [PAIRS, SQ, SK] u8
    mbt = np.ascontiguousarray(mb.transpose(0, 2, 1))  # [PAIRS, SK, SQ]

    in_maps = []
    for c in range(N_CORES):
        s = slice(c * PAIRS_PER_CORE, (c + 1) * PAIRS_PER_CORE)
        in_maps.append(
            {
                "qt": qt[s],
                "kt": kt[s],
                "v": vr[s],
                "mt": mbt[s],
            }
        )
    return in_maps


def kernel(query, key, value, dropout_mask):
    global LAST_RESULTS
    variant = os.environ.get("KERNEL_VARIANT", "v4")
    in_maps = _prep_inputs(query, key, value, dropout_mask, variant)
    nc = build_nc(variant=variant)
    res = run_bass_kernel_spmd(nc, in_maps, list(range(N_CORES)), trace=False)
    LAST_RESULTS = res
    outs = np.concatenate([r["ot"] for r in res.results], axis=0)  # [PAIRS, D, SQ]
    out = outs.astype(np.float32).transpose(0, 2, 1).reshape(B, H, SQ, D)
    return np.ascontiguousarray(out)

